# revision 9
# baseline (speedup 1.0000x reference)
"""Masked-softmax attention-scores kernel for Trainium2 (Bass/Tile), 8 cores.

Computes softmax(mask_fill(QK^T/sqrt(dk)) + syntax) for
q = query @ Wq.T + bq, k = key @ Wk.T + bk, heads split from d_model.

Sharding: 8 cores = 2 batches x 4 query-row quarters; every core handles all
12 heads for its (batch, row-slice).  Per core:
  - load Wq/Wk, PE-transpose them, project q rows + full key into
    head-transposed layout qT/kT [d_model x rows] (f32r matmuls),
  - per 128-row tile: comb = (mask*1e9 - 1e9) + syntax (DVE),
  - per head: scores matmul (K=64) + identity-matmul that adds comb into
    PSUM, ACT exp with fused row-sum, DVE reciprocal, normalize, DMA out.
Softmax is computed without max-subtraction: scores + syntax are O(10) here
(exp cannot overflow) and masked entries sit at ~-1e9 whose exp underflows
to exactly 0, matching the reference's -1e9 mask fill.
"""

from contextlib import ExitStack

import numpy as np

B, S, D, H = 2, 2048, 768, 12
DK = D // H
P = 128
NCORES = 8
RSPLIT = 4          # query-row splits per batch
R = S // RSPLIT     # query rows per core
NEG = -1.0e9


def build_program(S=S, D=D, H=H, R=R, mm_f32r=True, comb_via_pe=True):
    """Build the per-core SPMD Bass program (same program, 8 data shards)."""
    import concourse.bacc as bacc
    import concourse.mybir as mybir
    from concourse.masks import make_identity
    from concourse.tile import TileContext

    f32 = mybir.dt.float32
    i32 = mybir.dt.int32
    MMDT = mybir.dt.float32r if mm_f32r else f32
    ADD = mybir.AluOpType.add
    MULT = mybir.AluOpType.mult
    EXP = mybir.ActivationFunctionType.Exp
    COPY = mybir.ActivationFunctionType.Copy

    assert D % P == 0 and S % 512 == 0 and R % P == 0 and D // H == 64
    DC = D // P      # d-model chunks (6)
    RT = R // P      # query row tiles per core (4)
    NB = S // 512    # key-position blocks (4)

    nc = bacc.Bacc(trn_type="TRN2", target_bir_lowering=False, debug=False)

    q_in = nc.declare_dram_parameter("q_in", [R, D], f32, isOutput=False)
    k_in = nc.declare_dram_parameter("k_in", [S, D], f32, isOutput=False)
    syn = nc.declare_dram_parameter("syn", [R, S], f32, isOutput=False)
    msk = nc.declare_dram_parameter("msk", [R, S], i32, isOutput=False)
    wq = nc.declare_dram_parameter("wq", [D, D], f32, isOutput=False)
    bq = nc.declare_dram_parameter("bq", [D], f32, isOutput=False)
    wk = nc.declare_dram_parameter("wk", [D, D], f32, isOutput=False)
    bk = nc.declare_dram_parameter("bk", [D], f32, isOutput=False)
    out = nc.declare_dram_parameter("out", [H, R, S], f32, isOutput=True)

    with ExitStack() as ctx:
        tc = ctx.enter_context(TileContext(nc))

        consts = ctx.enter_context(tc.tile_pool(name="consts", bufs=1))
        ident = consts.tile([P, P], f32)          # for PE transposes (f32 in)
        make_identity(nc, ident)
        ident_r = consts.tile([P, P], MMDT)       # for comb-add matmul
        nc.vector.tensor_copy(ident_r, ident)
        ones_f = consts.tile([1, 512], f32)
        nc.vector.memset(ones_f, 1.0)
        ones = consts.tile([1, 512], MMDT)
        nc.vector.tensor_copy(ones, ones_f)

        # Persistent projected activations, head-transposed: qT/kT[dc] holds
        # d_model rows [dc*128, dc*128+128) x all s columns.
        persist = ctx.enter_context(tc.tile_pool(name="persist", bufs=1))
        kT = [persist.tile([P, S], MMDT, name=f"kT{i}", tag=f"kT{i}")
              for i in range(DC)]
        qT = [persist.tile([P, R], MMDT, name=f"qT{i}", tag=f"qT{i}")
              for i in range(DC)]

        # ---------------- prep (scoped: freed before the main loop) -------
        prep_ctx = ExitStack()
        wpool = prep_ctx.enter_context(tc.tile_pool(name="wprep", bufs=1))
        wqT = [wpool.tile([P, D], MMDT, name=f"wqT{i}", tag=f"wqT{i}")
               for i in range(DC)]
        wkT = [wpool.tile([P, D], MMDT, name=f"wkT{i}", tag=f"wkT{i}")
               for i in range(DC)]

        # Biases on a single partition; Wq/bq carry the 1/sqrt(dk) scale.
        bq_f = wpool.tile([1, D], f32, tag="bq_f")
        bk_f = wpool.tile([1, D], f32, tag="bk_f")
        nc.sync.dma_start(out=bq_f, in_=bq[None, :])
        nc.sync.dma_start(out=bk_f, in_=bk[None, :])
        bqs = wpool.tile([1, D], MMDT, tag="bqs")
        bks = wpool.tile([1, D], MMDT, tag="bks")
        nc.vector.tensor_scalar_mul(bqs, bq_f, 1.0 / 8.0)
        nc.vector.tensor_copy(bks, bk_f)

        with (
            tc.tile_pool(name="wraw", bufs=1) as wraw_pool,
            tc.tile_pool(name="tpp", bufs=2, space="PSUM") as tp_pool,
        ):
            for w_dram, wT, scale, wnm in ((wq, wqT, 1.0 / 8.0, "q"),
                                           (wk, wkT, 1.0, "k")):
                w_raws = []
                for di in range(DC):
                    w_raw = wraw_pool.tile([P, D], f32, name=f"wraw{wnm}{di}",
                                           tag=f"wraw{di}")
                    nc.sync.dma_start(out=w_raw, in_=w_dram[di * P:(di + 1) * P, :])
                    w_raws.append(w_raw)
                for fj in range(DC):
                    tp = tp_pool.tile([P, D], f32)
                    for di in range(DC):
                        nc.tensor.transpose(tp[:, di * P:(di + 1) * P],
                                            w_raws[di][:, fj * P:(fj + 1) * P],
                                            ident)
                    nc.vector.tensor_scalar_mul(wT[fj], tp, scale)

        # ---------------- prep: q transpose + projection ----------------
        with (
            tc.tile_pool(name="qraw", bufs=1) as qraw_pool,
            tc.tile_pool(name="qtr", bufs=1) as qtr_pool,
            tc.tile_pool(name="tpq", bufs=2, space="PSUM") as tpq_pool,
            tc.tile_pool(name="qproj", bufs=2, space="PSUM") as qproj_pool,
        ):
            qTraw = [qtr_pool.tile([P, R], MMDT, name=f"qTraw{i}", tag=f"qTraw{i}")
                     for i in range(DC)]
            q_raws = []
            for t in range(RT):
                q_raw = qraw_pool.tile([P, D], f32, name=f"qraw{t}",
                                       tag=f"qraw{t}")
                nc.sync.dma_start(out=q_raw, in_=q_in[t * P:(t + 1) * P, :])
                q_raws.append(q_raw)
            for fj in range(DC):
                tp = tpq_pool.tile([P, R], f32)
                for t in range(RT):
                    nc.tensor.transpose(tp[:, t * P:(t + 1) * P],
                                        q_raws[t][:, fj * P:(fj + 1) * P], ident)
                nc.vector.tensor_copy(qTraw[fj], tp)
            for dm in range(DC):
                for rb in range(max(1, R // 512)):
                    rw = min(512, R)
                    sl = slice(rb * 512, rb * 512 + rw)
                    ps = qproj_pool.tile([P, rw], f32)
                    for fj in range(DC):
                        nc.tensor.matmul(
                            ps, wqT[fj][:, dm * P:(dm + 1) * P], qTraw[fj][:, sl],
                            start=(fj == 0), stop=False,
                        )
                    nc.tensor.matmul(
                        ps, bqs[0:1, dm * P:(dm + 1) * P], ones[0:1, :rw],
                        start=False, stop=True,
                    )
                    nc.vector.tensor_copy(qT[dm][:, sl], ps)

        # ---------------- prep: key transpose + projection, streamed ------
        with (
            tc.tile_pool(name="kraw", bufs=5) as kraw_pool,
            tc.tile_pool(name="ktnb", bufs=2) as ktnb_pool,
            tc.tile_pool(name="tpk", bufs=2, space="PSUM") as tpk_pool,
            tc.tile_pool(name="kproj", bufs=1, space="PSUM") as kproj_pool,
        ):
            for nb in range(NB):
                k_raws = []
                for tt in range(4):
                    k_raw = kraw_pool.tile([P, D], f32)
                    row0 = nb * 512 + tt * P
                    nc.sync.dma_start(out=k_raw, in_=k_in[row0:row0 + P, :])
                    k_raws.append(k_raw)
                psk = [kproj_pool.tile([P, 512], f32, name=f"psk{dm}",
                                       tag=f"psk{dm}") for dm in range(DC)]
                for fj in range(DC):
                    kt_nb = ktnb_pool.tile([P, 512], MMDT)
                    tp = tpk_pool.tile([P, 512], f32)
                    for tt in range(4):
                        nc.tensor.transpose(tp[:, tt * P:(tt + 1) * P],
                                            k_raws[tt][:, fj * P:(fj + 1) * P],
                                            ident)
                    nc.vector.tensor_copy(kt_nb, tp)
                    for dm in range(DC):
                        nc.tensor.matmul(
                            psk[dm], wkT[fj][:, dm * P:(dm + 1) * P], kt_nb,
                            start=(fj == 0), stop=False,
                        )
                for dm in range(DC):
                    nc.tensor.matmul(
                        psk[dm], bks[0:1, dm * P:(dm + 1) * P], ones,
                        start=False, stop=True,
                    )
                    nc.vector.tensor_copy(kT[dm][:, nb * 512:(nb + 1) * 512], psk[dm])
        prep_ctx.close()

        # ---------------- main loop: scores + masked softmax --------------
        mskp = ctx.enter_context(tc.tile_pool(name="mskp", bufs=2))
        synp = ctx.enter_context(tc.tile_pool(name="synp", bufs=2))
        combp = ctx.enter_context(tc.tile_pool(name="combp", bufs=2))
        if not comb_via_pe:
            spool = ctx.enter_context(tc.tile_pool(name="spool", bufs=2))
        epool = ctx.enter_context(tc.tile_pool(name="epool", bufs=2))
        opool = ctx.enter_context(tc.tile_pool(name="opool", bufs=2))
        rpool = ctx.enter_context(tc.tile_pool(name="rpool", bufs=8))
        pspool = ctx.enter_context(tc.tile_pool(name="pspool", bufs=2, space="PSUM"))

        for t in range(RT):
            rows = slice(t * P, (t + 1) * P)
            # comb = (mask ? 0 : -1e9) + syntax
            msk_t = mskp.tile([P, S], i32)
            nc.sync.dma_start(out=msk_t, in_=msk[rows, :])
            syn_t = synp.tile([P, S], f32)
            nc.sync.dma_start(out=syn_t, in_=syn[rows, :])
            comb = combp.tile([P, S], MMDT if comb_via_pe else f32)
            nc.vector.tensor_scalar(comb, msk_t, 1.0e9, NEG, op0=MULT, op1=ADD)
            nc.vector.tensor_add(comb, comb, syn_t)

            for h in range(H):
                dc, off = h // 2, 64 * (h % 2)
                ps = pspool.tile([P, S], f32)
                for nb in range(NB):
                    cols = slice(nb * 512, (nb + 1) * 512)
                    nc.tensor.matmul(
                        ps[:, cols],
                        qT[dc][off:off + 64, rows],
                        kT[dc][off:off + 64, cols],
                        start=True, stop=not comb_via_pe,
                    )
                    if comb_via_pe:
                        nc.tensor.matmul(
                            ps[:, cols], ident_r, comb[:, cols],
                            start=False, stop=True,
                        )
                e = epool.tile([P, S], f32)
                rowsum = rpool.tile([P, 1], f32)
                if comb_via_pe:
                    nc.scalar.activation(e, ps, EXP, accum_out=rowsum)
                else:
                    s_t = spool.tile([P, S], f32, tag="s")
                    nc.vector.tensor_add(s_t, ps, comb)
                    nc.scalar.activation(e, s_t, EXP, accum_out=rowsum)
                rrec = rpool.tile([P, 1], f32)
                nc.vector.reciprocal(rrec, rowsum)
                o = opool.tile([P, S], f32)
                nc.any.tensor_scalar(o, e, scalar1=rrec, scalar2=None, op0=MULT)
                nc.sync.dma_start(out=out[h, rows, :], in_=o)

    nc.finalize()
    return nc


_NC_CACHE = {}


def _get_nc():
    key = "full"
    if key not in _NC_CACHE:
        _NC_CACHE[key] = build_program()
    return _NC_CACHE[key]


def shard_inputs(query, key, syntax_matrix, mask, Wq, bq, Wk, bk):
    in_maps = []
    for c in range(NCORES):
        b, r = divmod(c, RSPLIT)
        rows = slice(r * R, (r + 1) * R)
        in_maps.append({
            "q_in": np.ascontiguousarray(query[b, rows, :], np.float32),
            "k_in": np.ascontiguousarray(key[b], np.float32),
            "syn": np.ascontiguousarray(syntax_matrix[b, 0, rows, :], np.float32),
            "msk": np.ascontiguousarray(mask[b, rows, :], np.int32),
            "wq": np.ascontiguousarray(Wq, np.float32),
            "bq": np.ascontiguousarray(bq, np.float32),
            "wk": np.ascontiguousarray(Wk, np.float32),
            "bk": np.ascontiguousarray(bk, np.float32),
        })
    return in_maps


def assemble_output(results):
    out = np.empty((B, H, S, S), np.float32)
    for c in range(NCORES):
        b, r = divmod(c, RSPLIT)
        out[b, :, r * R:(r + 1) * R, :] = results[c]["out"]
    return out


def run_spmd(in_maps, **kwargs):
    from concourse.bass_utils import run_bass_kernel_spmd

    nc = _get_nc()
    return run_bass_kernel_spmd(nc, in_maps, list(range(NCORES)), **kwargs)


def kernel(query, key, vm, syntax_matrix, mask, Wq, bq, Wk, bk):
    query = np.asarray(query, np.float32)
    key = np.asarray(key, np.float32)
    syntax_matrix = np.asarray(syntax_matrix, np.float32)
    mask = np.asarray(mask, np.int32)
    Wq = np.asarray(Wq, np.float32)
    bq = np.asarray(bq, np.float32)
    Wk = np.asarray(Wk, np.float32)
    bk = np.asarray(bk, np.float32)

    in_maps = shard_inputs(query, key, syntax_matrix, mask, Wq, bq, Wk, bk)
    res = run_spmd(in_maps)
    return assemble_output(res.results)


# revision 10
# speedup vs baseline: 1.3017x; 1.3017x over previous
"""Masked-softmax attention-scores kernel for Trainium2 (Bass/Tile), 8 cores.

Computes softmax(mask_fill(QK^T/sqrt(dk)) + syntax) for
q = query @ Wq.T + bq, k = key @ Wk.T + bk, heads split from d_model.

Sharding: 8 cores = 2 batches x 4 query-row quarters; every core handles all
12 heads for its (batch, row-slice).  Per core:
  - load Wq/Wk, PE-transpose them, project q rows + full key into
    head-transposed layout qT/kT [d_model x rows] (f32r matmuls),
  - per 128-row tile: comb = (mask*1e9 - 1e9) + syntax (DVE),
  - per head: scores matmul (K=64) + identity-matmul that adds comb into
    PSUM, ACT exp with fused row-sum, DVE reciprocal, normalize, DMA out.
Softmax is computed without max-subtraction: scores + syntax are O(10) here
(exp cannot overflow) and masked entries sit at ~-1e9 whose exp underflows
to exactly 0, matching the reference's -1e9 mask fill.
"""

from contextlib import ExitStack

import numpy as np

B, S, D, H = 2, 2048, 768, 12
DK = D // H
P = 128
NCORES = 8
RSPLIT = 4          # query-row splits per batch
R = S // RSPLIT     # query rows per core
NEG = -1.0e9


def build_program(S=S, D=D, H=H, R=R, mm_f32r=True, comb_via_pe=False):
    """Build the per-core SPMD Bass program (same program, 8 data shards)."""
    import concourse.bacc as bacc
    import concourse.mybir as mybir
    from concourse.masks import make_identity
    from concourse.tile import TileContext

    f32 = mybir.dt.float32
    i32 = mybir.dt.int32
    MMDT = mybir.dt.float32r if mm_f32r else f32
    ADD = mybir.AluOpType.add
    MULT = mybir.AluOpType.mult
    EXP = mybir.ActivationFunctionType.Exp
    COPY = mybir.ActivationFunctionType.Copy

    assert D % P == 0 and S % 512 == 0 and R % P == 0 and D // H == 64
    DC = D // P      # d-model chunks (6)
    RT = R // P      # query row tiles per core (4)
    NB = S // 512    # key-position blocks (4)

    nc = bacc.Bacc(trn_type="TRN2", target_bir_lowering=False, debug=False)

    q_in = nc.declare_dram_parameter("q_in", [R, D], f32, isOutput=False)
    k_in = nc.declare_dram_parameter("k_in", [S, D], f32, isOutput=False)
    syn = nc.declare_dram_parameter("syn", [R, S], f32, isOutput=False)
    msk = nc.declare_dram_parameter("msk", [R, S], i32, isOutput=False)
    wq = nc.declare_dram_parameter("wq", [D, D], f32, isOutput=False)
    bq = nc.declare_dram_parameter("bq", [D], f32, isOutput=False)
    wk = nc.declare_dram_parameter("wk", [D, D], f32, isOutput=False)
    bk = nc.declare_dram_parameter("bk", [D], f32, isOutput=False)
    out = nc.declare_dram_parameter("out", [H, R, S], f32, isOutput=True)

    with ExitStack() as ctx:
        tc = ctx.enter_context(TileContext(nc))

        consts = ctx.enter_context(tc.tile_pool(name="consts", bufs=1))
        ident = consts.tile([P, P], f32)          # for PE transposes (f32 in)
        make_identity(nc, ident)
        ident_r = consts.tile([P, P], MMDT)       # for comb-add matmul
        nc.vector.tensor_copy(ident_r, ident)
        ones_f = consts.tile([1, 512], f32)
        nc.vector.memset(ones_f, 1.0)
        ones = consts.tile([1, 512], MMDT)
        nc.vector.tensor_copy(ones, ones_f)

        # Persistent projected activations, head-transposed: qT/kT[dc] holds
        # d_model rows [dc*128, dc*128+128) x all s columns.
        persist = ctx.enter_context(tc.tile_pool(name="persist", bufs=1))
        kT = [persist.tile([P, S], MMDT, name=f"kT{i}", tag=f"kT{i}")
              for i in range(DC)]
        qT = [persist.tile([P, R], MMDT, name=f"qT{i}", tag=f"qT{i}")
              for i in range(DC)]

        # ---------------- prep (scoped: freed before the main loop) -------
        prep_ctx = ExitStack()
        wpool = prep_ctx.enter_context(tc.tile_pool(name="wprep", bufs=1))
        wqT = [wpool.tile([P, D], MMDT, name=f"wqT{i}", tag=f"wqT{i}")
               for i in range(DC)]
        wkT = [wpool.tile([P, D], MMDT, name=f"wkT{i}", tag=f"wkT{i}")
               for i in range(DC)]

        # Biases on a single partition; Wq/bq carry the 1/sqrt(dk) scale.
        bq_f = wpool.tile([1, D], f32, tag="bq_f")
        bk_f = wpool.tile([1, D], f32, tag="bk_f")
        nc.sync.dma_start(out=bq_f, in_=bq[None, :])
        nc.sync.dma_start(out=bk_f, in_=bk[None, :])
        bqs = wpool.tile([1, D], MMDT, tag="bqs")
        bks = wpool.tile([1, D], MMDT, tag="bks")
        nc.vector.tensor_scalar_mul(bqs, bq_f, 1.0 / 8.0)
        nc.vector.tensor_copy(bks, bk_f)

        with (
            tc.tile_pool(name="wraw", bufs=1) as wraw_pool,
            tc.tile_pool(name="tpp", bufs=2, space="PSUM") as tp_pool,
        ):
            for w_dram, wT, scale, wnm in ((wq, wqT, 1.0 / 8.0, "q"),
                                           (wk, wkT, 1.0, "k")):
                w_raws = []
                for di in range(DC):
                    w_raw = wraw_pool.tile([P, D], f32, name=f"wraw{wnm}{di}",
                                           tag=f"wraw{di}")
                    nc.sync.dma_start(out=w_raw, in_=w_dram[di * P:(di + 1) * P, :])
                    w_raws.append(w_raw)
                for fj in range(DC):
                    tp = tp_pool.tile([P, D], f32)
                    for di in range(DC):
                        nc.tensor.transpose(tp[:, di * P:(di + 1) * P],
                                            w_raws[di][:, fj * P:(fj + 1) * P],
                                            ident)
                    nc.vector.tensor_scalar_mul(wT[fj], tp, scale)

        # ---------------- prep: q transpose + projection ----------------
        with (
            tc.tile_pool(name="qraw", bufs=1) as qraw_pool,
            tc.tile_pool(name="qtr", bufs=1) as qtr_pool,
            tc.tile_pool(name="tpq", bufs=2, space="PSUM") as tpq_pool,
            tc.tile_pool(name="qproj", bufs=2, space="PSUM") as qproj_pool,
        ):
            qTraw = [qtr_pool.tile([P, R], MMDT, name=f"qTraw{i}", tag=f"qTraw{i}")
                     for i in range(DC)]
            q_raws = []
            for t in range(RT):
                q_raw = qraw_pool.tile([P, D], f32, name=f"qraw{t}",
                                       tag=f"qraw{t}")
                nc.sync.dma_start(out=q_raw, in_=q_in[t * P:(t + 1) * P, :])
                q_raws.append(q_raw)
            for fj in range(DC):
                tp = tpq_pool.tile([P, R], f32)
                for t in range(RT):
                    nc.tensor.transpose(tp[:, t * P:(t + 1) * P],
                                        q_raws[t][:, fj * P:(fj + 1) * P], ident)
                nc.vector.tensor_copy(qTraw[fj], tp)
            for dm in range(DC):
                for rb in range(max(1, R // 512)):
                    rw = min(512, R)
                    sl = slice(rb * 512, rb * 512 + rw)
                    ps = qproj_pool.tile([P, rw], f32)
                    for fj in range(DC):
                        nc.tensor.matmul(
                            ps, wqT[fj][:, dm * P:(dm + 1) * P], qTraw[fj][:, sl],
                            start=(fj == 0), stop=False,
                        )
                    nc.tensor.matmul(
                        ps, bqs[0:1, dm * P:(dm + 1) * P], ones[0:1, :rw],
                        start=False, stop=True,
                    )
                    nc.vector.tensor_copy(qT[dm][:, sl], ps)

        # ---------------- prep: key transpose + projection, streamed ------
        with (
            tc.tile_pool(name="kraw", bufs=5) as kraw_pool,
            tc.tile_pool(name="ktnb", bufs=2) as ktnb_pool,
            tc.tile_pool(name="tpk", bufs=2, space="PSUM") as tpk_pool,
            tc.tile_pool(name="kproj", bufs=1, space="PSUM") as kproj_pool,
        ):
            for nb in range(NB):
                k_raws = []
                for tt in range(4):
                    k_raw = kraw_pool.tile([P, D], f32)
                    row0 = nb * 512 + tt * P
                    nc.sync.dma_start(out=k_raw, in_=k_in[row0:row0 + P, :])
                    k_raws.append(k_raw)
                psk = [kproj_pool.tile([P, 512], f32, name=f"psk{dm}",
                                       tag=f"psk{dm}") for dm in range(DC)]
                for fj in range(DC):
                    kt_nb = ktnb_pool.tile([P, 512], MMDT)
                    tp = tpk_pool.tile([P, 512], f32)
                    for tt in range(4):
                        nc.tensor.transpose(tp[:, tt * P:(tt + 1) * P],
                                            k_raws[tt][:, fj * P:(fj + 1) * P],
                                            ident)
                    nc.vector.tensor_copy(kt_nb, tp)
                    for dm in range(DC):
                        nc.tensor.matmul(
                            psk[dm], wkT[fj][:, dm * P:(dm + 1) * P], kt_nb,
                            start=(fj == 0), stop=False,
                        )
                for dm in range(DC):
                    nc.tensor.matmul(
                        psk[dm], bks[0:1, dm * P:(dm + 1) * P], ones,
                        start=False, stop=True,
                    )
                    nc.vector.tensor_copy(kT[dm][:, nb * 512:(nb + 1) * 512], psk[dm])
        prep_ctx.close()

        # ---------------- main loop: scores + masked softmax --------------
        mskp = ctx.enter_context(tc.tile_pool(name="mskp", bufs=2))
        synp = ctx.enter_context(tc.tile_pool(name="synp", bufs=2))
        combp = ctx.enter_context(tc.tile_pool(name="combp", bufs=2))
        if not comb_via_pe:
            spool = ctx.enter_context(tc.tile_pool(name="spool", bufs=2))
        epool = ctx.enter_context(tc.tile_pool(name="epool", bufs=2))
        opool = ctx.enter_context(tc.tile_pool(name="opool", bufs=2))
        rpool = ctx.enter_context(tc.tile_pool(name="rpool", bufs=8))
        pspool = ctx.enter_context(tc.tile_pool(name="pspool", bufs=2, space="PSUM"))

        for t in range(RT):
            rows = slice(t * P, (t + 1) * P)
            # comb = (mask ? 0 : -1e9) + syntax
            msk_t = mskp.tile([P, S], i32)
            nc.sync.dma_start(out=msk_t, in_=msk[rows, :])
            syn_t = synp.tile([P, S], f32)
            nc.sync.dma_start(out=syn_t, in_=syn[rows, :])
            comb = combp.tile([P, S], MMDT if comb_via_pe else f32)
            nc.gpsimd.tensor_scalar(comb, msk_t, 1.0e9, NEG, op0=MULT, op1=ADD)
            nc.gpsimd.tensor_add(comb, comb, syn_t)

            for h in range(H):
                dc, off = h // 2, 64 * (h % 2)
                ps = pspool.tile([P, S], f32)
                for nb in range(NB):
                    cols = slice(nb * 512, (nb + 1) * 512)
                    nc.tensor.matmul(
                        ps[:, cols],
                        qT[dc][off:off + 64, rows],
                        kT[dc][off:off + 64, cols],
                        start=True, stop=not comb_via_pe,
                    )
                    if comb_via_pe:
                        nc.tensor.matmul(
                            ps[:, cols], ident_r, comb[:, cols],
                            start=False, stop=True,
                        )
                e = epool.tile([P, S], f32)
                rowsum = rpool.tile([P, 1], f32)
                if comb_via_pe:
                    nc.scalar.activation(e, ps, EXP, accum_out=rowsum)
                else:
                    s_t = spool.tile([P, S], f32, tag="s")
                    nc.vector.tensor_add(s_t, ps, comb)
                    nc.scalar.activation(e, s_t, EXP, accum_out=rowsum)
                rrec = rpool.tile([P, 1], f32)
                nc.vector.reciprocal(rrec, rowsum)
                o = opool.tile([P, S], f32)
                if h % 2 == 0:
                    nc.vector.tensor_scalar(o, e, scalar1=rrec, scalar2=None,
                                            op0=MULT)
                else:
                    nc.scalar.activation(o, e, COPY, bias=0.0, scale=rrec)
                nc.sync.dma_start(out=out[h, rows, :], in_=o)

    nc.finalize()
    return nc


_NC_CACHE = {}


def _get_nc():
    key = "full"
    if key not in _NC_CACHE:
        _NC_CACHE[key] = build_program()
    return _NC_CACHE[key]


def shard_inputs(query, key, syntax_matrix, mask, Wq, bq, Wk, bk):
    in_maps = []
    for c in range(NCORES):
        b, r = divmod(c, RSPLIT)
        rows = slice(r * R, (r + 1) * R)
        in_maps.append({
            "q_in": np.ascontiguousarray(query[b, rows, :], np.float32),
            "k_in": np.ascontiguousarray(key[b], np.float32),
            "syn": np.ascontiguousarray(syntax_matrix[b, 0, rows, :], np.float32),
            "msk": np.ascontiguousarray(mask[b, rows, :], np.int32),
            "wq": np.ascontiguousarray(Wq, np.float32),
            "bq": np.ascontiguousarray(bq, np.float32),
            "wk": np.ascontiguousarray(Wk, np.float32),
            "bk": np.ascontiguousarray(bk, np.float32),
        })
    return in_maps


def assemble_output(results):
    out = np.empty((B, H, S, S), np.float32)
    for c in range(NCORES):
        b, r = divmod(c, RSPLIT)
        out[b, :, r * R:(r + 1) * R, :] = results[c]["out"]
    return out


def run_spmd(in_maps, **kwargs):
    from concourse.bass_utils import run_bass_kernel_spmd

    nc = _get_nc()
    return run_bass_kernel_spmd(nc, in_maps, list(range(NCORES)), **kwargs)


def kernel(query, key, vm, syntax_matrix, mask, Wq, bq, Wk, bk):
    query = np.asarray(query, np.float32)
    key = np.asarray(key, np.float32)
    syntax_matrix = np.asarray(syntax_matrix, np.float32)
    mask = np.asarray(mask, np.int32)
    Wq = np.asarray(Wq, np.float32)
    bq = np.asarray(bq, np.float32)
    Wk = np.asarray(Wk, np.float32)
    bk = np.asarray(bk, np.float32)

    in_maps = shard_inputs(query, key, syntax_matrix, mask, Wq, bq, Wk, bk)
    res = run_spmd(in_maps)
    return assemble_output(res.results)
